# revision 7
# baseline (speedup 1.0000x reference)
"""Trainium2 Bass kernel for nn_AdvResNet (dense_mlp, 8 NeuronCores).

Reference math (adv=1 path, the one setup_inputs produces):
    beta_norm[n] = sum_k |beta[k, n]|                       # [1024]
    one[n]      = 4096 * sum_h W2[n, h] + bias2[n]          # [1024]
    out[b, n]   = (x @ beta)[b, n] + bias_lin[n]
                  - 0.1 * y[b, n] * beta_norm[n] + one[n]

The x@W1 relu MLP is dead code when adv=1, so W1/bias1 never touch the
device.

Distribution: data-parallel over batch (512 rows/core), beta replicated.
Each core computes in TRANSPOSED layout: outT = beta^T @ x^T via
matmul(psum[n,b], lhsT=beta[k,n] (natural layout), rhs=xT[k,b]), so the
per-n vectors (beta_norm, one, biases) are per-partition scalars, which
feed the scalar-engine activation(scale*in+bias) directly.

The beta_norm / W2-rowsum reductions are sharded 8-ways (each core
reduces a 1/8 slice along the contraction axis with a free-axis
vector-reduce, abs fused) and combined with a single 8KB AllReduce.

Matmuls run in float32r (fp32 operands, 1 cycle/row at N=512).
"""

import os
import sys

sys.path.insert(0, "/opt/trn_rl_repo")
os.environ.setdefault("NEURON_RT_RESET_CORES", "1")

import numpy as np

import concourse.bass as bass  # noqa: F401
import concourse.tile as tile
from concourse import bacc, mybir
from concourse.bass_utils import run_bass_kernel_spmd

B, NIN, NHID, NOUT = 4096, 2048, 4096, 1024
NC = 8
BS = B // NC  # 512 batch rows per core
KT = NIN // 128  # 16 k-tiles
NT = NOUT // 128  # 8 n-tiles
KSH = NIN // NC  # 256: beta_norm k-slice per core
HSH = NHID // NC  # 512: W2 h-slice per core
EPS = 0.1
F32 = mybir.dt.float32
F32R = mybir.dt.float32r

_CACHE = {}


def build_bass():
    nc = bacc.Bacc("TRN2", target_bir_lowering=False, debug=False, num_devices=NC)

    xT = nc.declare_dram_parameter("xT", [NIN, BS], F32, isOutput=False)
    yT = nc.declare_dram_parameter("yT", [NOUT, BS], F32, isOutput=False)
    bet = nc.declare_dram_parameter("beta", [NIN, NOUT], F32, isOutput=False)
    btp = nc.declare_dram_parameter("btp", [128, NT, KSH], F32, isOutput=False)
    w2p = nc.declare_dram_parameter("w2p", [128, NT, HSH], F32, isOutput=False)
    blp = nc.declare_dram_parameter("blp", [128, NT], F32, isOutput=False)
    b2p = nc.declare_dram_parameter("b2p", [128, NT], F32, isOutput=False)
    out = nc.declare_dram_parameter("out", [NOUT, BS], F32, isOutput=True)

    with (
        tile.TileContext(nc) as tc,
        tc.tile_pool(name="bsb", bufs=KT) as bpool,
        tc.tile_pool(name="xsb", bufs=KT) as xpool,
        tc.tile_pool(name="yts", bufs=NT) as ypool,
        tc.tile_pool(name="aux", bufs=1) as aux,
        tc.tile_pool(name="psum", bufs=1, space="PSUM") as ppool,
        tc.tile_pool(name="dram", bufs=1, space="DRAM") as dpool,
    ):
        ps = [
            ppool.tile([128, BS], F32, name=f"ps{n}", tag=f"ps{n}")
            for n in range(NT)
        ]

        # ---- Collective path: its 3MB of inputs get the FIRST bytes on the
        # sync ring, chunked so the reduces pipeline with the DMAs, and the
        # bounce DMAs + AllReduce live on the otherwise-idle gpsimd engine
        # (own SWDGE sem lane, never queued behind stream DMAs).  This makes
        # the trigger fire ~15us into each core's local time so the
        # cross-core start skew hides under the matmul stream.
        H2 = NT // 2
        w2s = aux.tile([128, NT, HSH], F32)
        bts = aux.tile([128, NT, KSH], F32)
        part = aux.tile([128, 2 * NT], F32)
        for h in (0, 1):
            sl = slice(h * H2, (h + 1) * H2)
            nc.sync.dma_start(out=w2s[:, sl, :], in_=w2p[:, sl, :])
            nc.vector.tensor_reduce(
                out=part[:, h * H2 : (h + 1) * H2],
                in_=w2s[:, sl, :],
                axis=mybir.AxisListType.X,
                op=mybir.AluOpType.add,
            )
        for h in (0, 1):
            sl = slice(h * H2, (h + 1) * H2)
            nc.sync.dma_start(out=bts[:, sl, :], in_=btp[:, sl, :])
            nc.vector.tensor_reduce(
                out=part[:, NT + h * H2 : NT + (h + 1) * H2],
                in_=bts[:, sl, :],
                axis=mybir.AxisListType.X,
                op=mybir.AluOpType.add,
                apply_absolute_value=True,
            )
        cin = dpool.tile([128, 2 * NT], F32)
        cout = dpool.tile([128, 2 * NT], F32)
        nc.gpsimd.dma_start(out=cin[:], in_=part[:])
        nc.gpsimd.collective_compute(
            "AllReduce",
            mybir.AluOpType.add,
            replica_groups=[list(range(NC))],
            ins=[cin.opt()],
            outs=[cout.opt()],
        )
        allred = aux.tile([128, 2 * NT], F32)
        nc.gpsimd.dma_start(out=allred[:], in_=cout[:])

        # scale[n] = -EPS * beta_norm[n];  biasc[n] = NHID*w2sum + bias2 + bias_lin
        blt = aux.tile([128, NT], F32)
        nc.scalar.dma_start(out=blt[:], in_=blp[:])
        b2t = aux.tile([128, NT], F32)
        nc.scalar.dma_start(out=b2t[:], in_=b2p[:])
        scale = aux.tile([128, NT], F32)
        nc.vector.tensor_scalar_mul(scale[:], allred[:, NT : 2 * NT], -EPS)
        biasc = aux.tile([128, NT], F32)
        nc.vector.tensor_scalar_mul(biasc[:], allred[:, 0:NT], float(NHID))
        nc.vector.tensor_add(biasc[:], biasc[:], b2t[:])
        nc.vector.tensor_add(biasc[:], biasc[:], blt[:])

        # ---- Main matmul stream: k-outer / n-inner, beta+xT on the sync
        # ring, uninterrupted so the PE never idles (HAM stays warm).  yT
        # rides the sync ring mid-stream; its ACT precompute
        # t[n] = yT*scale + biasc runs as soon as the collective lands.
        yts = []
        for k in range(KT):
            bt = bpool.tile([128, NOUT], F32R, tag="bt")
            nc.sync.dma_start(
                out=bt[:], in_=bet[k * 128 : (k + 1) * 128, :].bitcast(F32R)
            )
            xt = xpool.tile([128, BS], F32R, tag="xt")
            nc.sync.dma_start(
                out=xt[:], in_=xT[k * 128 : (k + 1) * 128, :].bitcast(F32R)
            )
            for n in range(NT):
                nc.tensor.matmul(
                    ps[n][:],
                    lhsT=bt[:, n * 128 : (n + 1) * 128],
                    rhs=xt[:],
                    start=(k == 0),
                    stop=(k == KT - 1),
                )
            if k == KT // 2:
                for n in range(NT):
                    yt = ypool.tile([128, BS], F32, tag="yt", name=f"yt{n}")
                    nc.sync.dma_start(
                        out=yt[:], in_=yT[n * 128 : (n + 1) * 128, :]
                    )
                    nc.scalar.activation(
                        yt[:],
                        yt[:],
                        mybir.ActivationFunctionType.Identity,
                        bias=biasc[:, n : n + 1],
                        scale=scale[:, n : n + 1],
                    )
                    yts.append(yt)

        # Epilogue: out = psum(lin^T) + t, then store on the scalar ring.
        for n in range(NT):
            nc.vector.tensor_add(yts[n][:], ps[n][:], yts[n][:])
            nc.scalar.dma_start(out=out[n * 128 : (n + 1) * 128, :], in_=yts[n][:])

    nc.compile()
    return nc


def _get_nc():
    if "nc" not in _CACHE:
        _CACHE["nc"] = build_bass()
    return _CACHE["nc"]


def _shard_inputs(x, y, beta, bias_lin, W2, bias2):
    x = np.ascontiguousarray(x, dtype=np.float32)
    y = np.ascontiguousarray(y, dtype=np.float32)
    beta = np.ascontiguousarray(beta, dtype=np.float32)
    W2 = np.ascontiguousarray(W2, dtype=np.float32)
    blp = np.ascontiguousarray(np.asarray(bias_lin, np.float32).reshape(NT, 128).T)
    b2p = np.ascontiguousarray(np.asarray(bias2, np.float32).reshape(NT, 128).T)
    betaT = np.ascontiguousarray(beta.T)  # [NOUT, NIN]
    in_maps = []
    for c in range(NC):
        bsl = slice(c * BS, (c + 1) * BS)
        # [128, NT, KSH]: btp[p, t, k] = |slice later| beta[c*KSH+k, t*128+p]
        btp = np.ascontiguousarray(
            betaT[:, c * KSH : (c + 1) * KSH]
            .reshape(NT, 128, KSH)
            .transpose(1, 0, 2)
        )
        w2p = np.ascontiguousarray(
            W2[:, c * HSH : (c + 1) * HSH].reshape(NT, 128, HSH).transpose(1, 0, 2)
        )
        in_maps.append(
            {
                "xT": np.ascontiguousarray(x[bsl].T),
                "yT": np.ascontiguousarray(y[bsl].T),
                "beta": beta,
                "btp": btp,
                "w2p": w2p,
                "blp": blp,
                "b2p": b2p,
            }
        )
    return in_maps


def run_device(inputs, trace=False, **kw):
    nc = _get_nc()
    in_maps = _shard_inputs(
        inputs["x"], inputs["y"], inputs["beta"], inputs["bias_lin"],
        inputs["W2"], inputs["bias2"],
    )
    res = run_bass_kernel_spmd(nc, in_maps, core_ids=list(range(NC)), trace=trace, **kw)
    full = np.empty((B, NOUT), dtype=np.float32)
    for c in range(NC):
        full[c * BS : (c + 1) * BS, :] = res.results[c]["out"].T
    return full, res


def _reference_numpy(x, y, beta, bias_lin, W1, W2, bias1, bias2, adv):
    # Fallback for the adv=0 path (never produced by setup_inputs).
    x = np.asarray(x, np.float32)
    lin = x @ np.asarray(beta, np.float32) + np.asarray(bias_lin, np.float32)
    if adv:
        beta_norm = np.sum(np.abs(np.asarray(beta, np.float32)), axis=0)
        lin = lin - EPS * np.asarray(y, np.float32) * beta_norm
        one = NHID * np.sum(np.asarray(W2, np.float32), axis=1) + np.asarray(
            bias2, np.float32
        )
        one = np.broadcast_to(one, lin.shape)
    else:
        h = np.maximum(
            x @ np.asarray(W1, np.float32).T + np.asarray(bias1, np.float32), 0.0
        )
        one = h @ np.asarray(W2, np.float32).T + np.asarray(bias2, np.float32)
    return (lin + one).astype(np.float32)


def kernel(**inputs) -> np.ndarray:
    adv = int(np.asarray(inputs.get("adv", 1)))
    if adv == 0:
        return _reference_numpy(
            inputs["x"], inputs["y"], inputs["beta"], inputs["bias_lin"],
            inputs["W1"], inputs["W2"], inputs["bias1"], inputs["bias2"], adv,
        )
    full, _ = run_device(inputs)
    return full


# revision 8
# speedup vs baseline: 1.0255x; 1.0255x over previous
"""Trainium2 Bass kernel for nn_AdvResNet (dense_mlp, 8 NeuronCores).

Reference math (adv=1 path, the one setup_inputs produces):
    beta_norm[n] = sum_k |beta[k, n]|                       # [1024]
    one[n]      = 4096 * sum_h W2[n, h] + bias2[n]          # [1024]
    out[b, n]   = (x @ beta)[b, n] + bias_lin[n]
                  - 0.1 * y[b, n] * beta_norm[n] + one[n]

The x@W1 relu MLP is dead code when adv=1, so W1/bias1 never touch the
device.

Distribution: a 2 (n-halves) x 4 (batch-quarters) grid with ZERO
collectives — collective_compute costs ~73us of latency in this
environment (measured on a bare 8KB AllReduce), far more than the extra
DMA this layout pays.  Core c = (h = c%2, g = c//2) computes
outT[h-half n (512), batch-quarter g (1024 b)].  Every per-n quantity
is then core-local: beta_norm from the core's own beta[:, n-half]
(abs-accumulated on ACT/DVE while the matmul streams), and one[n] from
the core's own W2[n-half, :] rows (free-axis vector reduce).

Compute is in TRANSPOSED layout: outT = beta^T @ x^T via
matmul(psum[n,b], lhsT=beta[k,n] (natural layout), rhs=xT[k,b]), so the
per-n vectors (beta_norm, one, biases) are per-partition scalars, which
feed the scalar-engine activation(scale*in+bias) directly.

Matmuls run in float32r (fp32 operands, 1 cycle/row at N=512).
"""

import os
import sys

sys.path.insert(0, "/opt/trn_rl_repo")
os.environ.setdefault("NEURON_RT_RESET_CORES", "1")

import numpy as np

import concourse.bass as bass  # noqa: F401
import concourse.bass_isa as bass_isa
import concourse.tile as tile
from concourse import bacc, mybir
from concourse.bass_utils import run_bass_kernel_spmd

B, NIN, NHID, NOUT = 4096, 2048, 4096, 1024
NC = 8
PN, PB = 2, 4  # core grid: n-halves x batch-quarters
NH = NOUT // PN  # 512 n per core
BSH = B // PB  # 1024 batch rows per core
KT = NIN // 128  # 16 k-tiles
NT = NH // 128  # 4 n-tiles per core
W2C = 4  # W2 h-chunks streamed per core
EPS = 0.1
F32 = mybir.dt.float32
F32R = mybir.dt.float32r

_CACHE = {}


def build_bass():
    nc = bacc.Bacc("TRN2", target_bir_lowering=False, debug=False, num_devices=NC)

    xT = nc.declare_dram_parameter("xT", [NIN, BSH], F32, isOutput=False)
    yT = nc.declare_dram_parameter("yT", [NH, BSH], F32, isOutput=False)
    bet = nc.declare_dram_parameter("beta", [NIN, NH], F32, isOutput=False)
    w2p = nc.declare_dram_parameter("w2p", [128, NT, NHID], F32, isOutput=False)
    blp = nc.declare_dram_parameter("blp", [128, NT], F32, isOutput=False)
    b2p = nc.declare_dram_parameter("b2p", [128, NT], F32, isOutput=False)
    out = nc.declare_dram_parameter("out", [NH, BSH], F32, isOutput=True)

    HC = NHID // W2C

    with (
        tile.TileContext(nc) as tc,
        tc.tile_pool(name="bsb", bufs=KT) as bpool,
        tc.tile_pool(name="xsb", bufs=KT) as xpool,
        tc.tile_pool(name="yts", bufs=NT) as ypool,
        tc.tile_pool(name="absb", bufs=2) as apool,
        tc.tile_pool(name="w2b", bufs=2) as wpool,
        tc.tile_pool(name="aux", bufs=1) as aux,
        tc.tile_pool(name="psum", bufs=1, space="PSUM") as ppool,
        tc.tile_pool(name="dram", bufs=1, space="DRAM") as dpool,
    ):
        ps = [
            [
                ppool.tile([128, 512], F32, name=f"ps{t}_{j}", tag=f"ps{t}_{j}")
                for j in range(2)
            ]
            for t in range(NT)
        ]
        acc = aux.tile([128, NH], F32)
        w2acc = aux.tile([128, NT], F32)

        def k_step(k):
            bt = bpool.tile([128, NH], F32R, tag="bt")
            nc.sync.dma_start(
                out=bt[:], in_=bet[k * 128 : (k + 1) * 128, :].bitcast(F32R)
            )
            xt = xpool.tile([128, BSH], F32R, tag="xt")
            nc.sync.dma_start(
                out=xt[:], in_=xT[k * 128 : (k + 1) * 128, :].bitcast(F32R)
            )
            for t in range(NT):
                for j in range(2):
                    nc.tensor.matmul(
                        ps[t][j][:],
                        lhsT=bt[:, t * 128 : (t + 1) * 128],
                        rhs=xt[:, j * 512 : (j + 1) * 512],
                        start=(k == 0),
                        stop=(k == KT - 1),
                    )
            # |beta| accumulation for beta_norm rides along on ACT + DVE.
            ab = apool.tile([128, NH], F32, tag="ab")
            nc.scalar.activation(
                ab[:], bt[:].bitcast(F32), mybir.ActivationFunctionType.Abs
            )
            if k == 0:
                nc.vector.tensor_copy(acc[:], ab[:])
            else:
                nc.vector.tensor_add(acc[:], acc[:], ab[:])

        def w2_step(c):
            wt = wpool.tile([128, NT, HC], F32, tag="wt")
            nc.sync.dma_start(out=wt[:], in_=w2p[:, :, c * HC : (c + 1) * HC])
            pr = aux.tile([128, NT], F32, name=f"w2pr{c}", tag=f"w2pr{c}")
            nc.vector.tensor_reduce(
                out=pr[:],
                in_=wt[:],
                axis=mybir.AxisListType.X,
                op=mybir.AluOpType.add,
            )
            if c == 0:
                nc.vector.tensor_copy(w2acc[:], pr[:])
            else:
                nc.vector.tensor_add(w2acc[:], w2acc[:], pr[:])

        # Stream: beta/xT k-tiles with W2 chunks interleaved mid-stream so
        # the DVE reduces overlap the matmuls instead of the epilogue.
        for k in range(KT):
            k_step(k)
            if k in (3, 5, 7, 9):
                w2_step((k - 3) // 2)
            if k == 11:
                # bias/one precompute (local, no cross-core anything):
                # biasc = NHID*w2sum + bias2 + bias_lin
                blt = aux.tile([128, NT], F32)
                nc.scalar.dma_start(out=blt[:], in_=blp[:])
                b2t = aux.tile([128, NT], F32)
                nc.scalar.dma_start(out=b2t[:], in_=b2p[:])
                biasc = aux.tile([128, NT], F32)
                nc.vector.tensor_scalar_mul(biasc[:], w2acc[:], float(NHID))
                nc.vector.tensor_add(biasc[:], biasc[:], b2t[:])
                nc.vector.tensor_add(biasc[:], biasc[:], blt[:])
                yts = []
                for t in range(NT):
                    yt = ypool.tile([128, BSH], F32, tag="yt", name=f"yt{t}")
                    nc.scalar.dma_start(
                        out=yt[:], in_=yT[t * 128 : (t + 1) * 128, :]
                    )
                    yts.append(yt)

        # beta_norm: partition-reduce acc on the idle gpsimd engine, then a
        # 2KB DRAM round-trip to land it as per-partition columns.
        accr = aux.tile([128, NH], F32)
        nc.gpsimd.partition_all_reduce(
            accr[:], acc[:], channels=128, reduce_op=bass_isa.ReduceOp.add
        )
        bscr = dpool.tile([NT, 128], F32)
        nc.gpsimd.dma_start(out=bscr[:], in_=accr[0:1, :])
        bnc = aux.tile([128, NT], F32)
        nc.gpsimd.dma_start(out=bnc[:], in_=bscr[:].rearrange("t p -> p t"))
        scale = aux.tile([128, NT], F32)
        nc.vector.tensor_scalar_mul(scale[:], bnc[:], -EPS)

        # Epilogue: t = yT*scale + biasc (ACT), out = psum + t (DVE), store.
        for t in range(NT):
            nc.scalar.activation(
                yts[t][:],
                yts[t][:],
                mybir.ActivationFunctionType.Identity,
                bias=biasc[:, t : t + 1],
                scale=scale[:, t : t + 1],
            )
            for j in range(2):
                sl = slice(j * 512, (j + 1) * 512)
                nc.vector.tensor_add(yts[t][:, sl], ps[t][j][:], yts[t][:, sl])
            nc.sync.dma_start(out=out[t * 128 : (t + 1) * 128, :], in_=yts[t][:])

    nc.compile()
    return nc


def _get_nc():
    if "nc" not in _CACHE:
        _CACHE["nc"] = build_bass()
    return _CACHE["nc"]


def _shard_inputs(x, y, beta, bias_lin, W2, bias2):
    x = np.ascontiguousarray(x, dtype=np.float32)
    y = np.ascontiguousarray(y, dtype=np.float32)
    beta = np.ascontiguousarray(beta, dtype=np.float32)
    W2 = np.ascontiguousarray(W2, dtype=np.float32)
    bias_lin = np.asarray(bias_lin, np.float32)
    bias2 = np.asarray(bias2, np.float32)
    xT_full = np.ascontiguousarray(x.T)  # [NIN, B]
    xT_g = [
        np.ascontiguousarray(xT_full[:, g * BSH : (g + 1) * BSH]) for g in range(PB)
    ]
    beta_h = [
        np.ascontiguousarray(beta[:, h * NH : (h + 1) * NH]) for h in range(PN)
    ]
    w2p_h = [
        np.ascontiguousarray(
            W2[h * NH : (h + 1) * NH, :].reshape(NT, 128, NHID).transpose(1, 0, 2)
        )
        for h in range(PN)
    ]
    blp_h = [
        np.ascontiguousarray(bias_lin[h * NH : (h + 1) * NH].reshape(NT, 128).T)
        for h in range(PN)
    ]
    b2p_h = [
        np.ascontiguousarray(bias2[h * NH : (h + 1) * NH].reshape(NT, 128).T)
        for h in range(PN)
    ]
    in_maps = []
    for c in range(NC):
        h, g = c % PN, c // PN
        yT = np.ascontiguousarray(
            y[g * BSH : (g + 1) * BSH, h * NH : (h + 1) * NH].T
        )
        in_maps.append(
            {
                "xT": xT_g[g],
                "yT": yT,
                "beta": beta_h[h],
                "w2p": w2p_h[h],
                "blp": blp_h[h],
                "b2p": b2p_h[h],
            }
        )
    return in_maps


def run_device(inputs, trace=False, **kw):
    nc = _get_nc()
    in_maps = _shard_inputs(
        inputs["x"], inputs["y"], inputs["beta"], inputs["bias_lin"],
        inputs["W2"], inputs["bias2"],
    )
    res = run_bass_kernel_spmd(nc, in_maps, core_ids=list(range(NC)), trace=trace, **kw)
    full = np.empty((B, NOUT), dtype=np.float32)
    for c in range(NC):
        h, g = c % PN, c // PN
        full[g * BSH : (g + 1) * BSH, h * NH : (h + 1) * NH] = res.results[c][
            "out"
        ].T
    return full, res


def _reference_numpy(x, y, beta, bias_lin, W1, W2, bias1, bias2, adv):
    # Fallback for the adv=0 path (never produced by setup_inputs).
    x = np.asarray(x, np.float32)
    lin = x @ np.asarray(beta, np.float32) + np.asarray(bias_lin, np.float32)
    if adv:
        beta_norm = np.sum(np.abs(np.asarray(beta, np.float32)), axis=0)
        lin = lin - EPS * np.asarray(y, np.float32) * beta_norm
        one = NHID * np.sum(np.asarray(W2, np.float32), axis=1) + np.asarray(
            bias2, np.float32
        )
        one = np.broadcast_to(one, lin.shape)
    else:
        h = np.maximum(
            x @ np.asarray(W1, np.float32).T + np.asarray(bias1, np.float32), 0.0
        )
        one = h @ np.asarray(W2, np.float32).T + np.asarray(bias2, np.float32)
    return (lin + one).astype(np.float32)


def kernel(**inputs) -> np.ndarray:
    adv = int(np.asarray(inputs.get("adv", 1)))
    if adv == 0:
        return _reference_numpy(
            inputs["x"], inputs["y"], inputs["beta"], inputs["bias_lin"],
            inputs["W1"], inputs["W2"], inputs["bias1"], inputs["bias2"], adv,
        )
    full, _ = run_device(inputs)
    return full
